# revision 19
# baseline (speedup 1.0000x reference)
"""Multi-head attention (B=2, H=16, S=2048, D=64) on 8 trn2 NeuronCores.

Sharding: the 32 (b, h) head-units are split 4-per-core (head/data parallel,
no cross-core comms).  Per core, for each head:

  scoresT[k, q] = sum_d K[k, d] Q[q, d] / 8        (PE, contract=64, row-packed 2x)
  pT[k, q]      = exp(scoresT) * keep01T[k, q]     (ACT exp fused w/ scale + psum
                                                    evacuation; DVE fp16 mask mul)
  OT'[m, q]     = sum_k V'[k, m] pT[k, q]          (PE, V' = [V | ones] so row 64
                                                    of OT' is the softmax denom Z)
  out[q, d]     = OT'[d, q] / OT'[64, q]           (host-side: O(S*D) divide +
                                                    transpose while unsharding)

Working in the transposed-score layout means softmax needs no reductions at
all (Z rides along in the PV matmul) and no S x S transposes anywhere.

Host-side (numpy, not on the critical HW path): Q/K are passed pre-transposed
per head as [64, S]; V is passed chunk-interleaved fp16 with the ones column
appended; the shared mask is passed transposed as a 0/1 fp16 matrix.
"""

import numpy as np

import concourse.bass as bass  # noqa: F401  (engine types resolve through nc)
import concourse.mybir as mybir
import concourse.tile as tile
from concourse import bacc
from concourse.bass_utils import run_bass_kernel_spmd

B, H, S, D = 2, 16, 2048, 64
N_CORES = 8
HPC = (B * H) // N_CORES  # heads per core

SQ = 512        # query-block width (one fp32 PSUM bank)
CK = 128        # key-chunk height (PSUM partition dim)
# Key chunks per exp group: 3-bank [128, 1536] PSUM groups maximize the ACT
# call size (per-ACTIVATE overhead is ~0.4us on HW) within the 8-bank budget
# (2x 3-bank qk slots + 2x 1-bank PV accumulators).
GROUPS = [(0, 3), (3, 3), (6, 3), (9, 3), (12, 3), (15, 1)]
HALVES = [(0, 0, 2), (1, 2, 6)]   # (half idx, first group, end group)
VW = D + 2      # V' width: 64 V columns + ones column + pad (66)

# Hybrid exp split: the last n_dve key-chunks of every unit evacuate PSUM via
# a DVE Schraudolph pass instead of ACT exp, balancing the two engines:
#   i16 = round(A*s + B); bitcast fp16  =>  ~2^(0.125*log2e*s) = exp(s/8)
# (max elementwise rel err ~3%, which softmax-averages to ~9e-3 end-to-end
# on this problem's flat score distribution).
EXP_A = 128 * np.log2(np.e)       # 1024 * log2(e) * 0.125
EXP_B = 15360.0 - 45.0            # fp16 exp bias 15<<10, minus minimax tweak


def make_groups(n_dve):
    """(c0, n, path) group list: 'a' = ACT exp, 'd' = DVE Schraudolph."""
    if n_dve == 0:
        return [(c0, n, "a") for c0, n in GROUPS]
    n_act = 16 - n_dve
    gs = []
    c = 0
    while c < n_act:
        n = min(3, n_act - c)
        gs.append((c, n, "a"))
        c += n
    while c < 16:
        n = min(3, 16 - c)
        gs.append((c, n, "d"))
        c += n
    return gs

f32 = mybir.dt.float32
f16 = mybir.dt.float16
FT = mybir.ActivationFunctionType


def build_nc(hpc=HPC, s=S, loop_n=None, ablate=(), loop_stagger=False,
             stagger=2, n_dve=0, tail_act=False, n_gp=0, qk128=False):
    """Build the per-core Bass program (identical on all 8 cores).

    loop_n: if set, wrap the whole body in an on-device For_i loop that
    recomputes the same output loop_n times — a perf-measurement rig that
    lets wall-clock deltas between two loop_n values cancel host/RPC
    overheads (this container has no NTFF profile path).

    ablate: perf-debug only — subset of {"qk", "act", "mask", "pv", "tail"}
    to skip emitting, isolating per-engine throughput on HW. Output is
    garbage when non-empty.

    stagger: software-pipeline depth in half-units — half k's QK/exp/mask
    stream is emitted alongside the PV matmuls of half k-stagger.

    n_dve: number of key-chunks per unit whose exp runs on the DVE
    (Schraudolph bitcast) instead of ACT, balancing the two engines.
    """
    nsq = s // SQ
    nck = s // CK
    groups = make_groups(n_dve) if nck == 16 else [
        (c, 1, "a") for c in range(nck)]
    ablate = set(ablate)

    nc = bacc.Bacc("TRN2", target_bir_lowering=False, debug=False)

    qt_d = nc.dram_tensor("qt", [hpc, D, s], f16, kind="ExternalInput")
    kt_d = nc.dram_tensor("kt", [hpc, D, s], f16, kind="ExternalInput")
    vp_d = nc.dram_tensor("vp", [hpc, CK, nck * VW], f16, kind="ExternalInput")
    mk_d = nc.dram_tensor("mk", [nsq, CK, nck * SQ], f16, kind="ExternalInput")
    o_d = nc.dram_tensor("o", [hpc, nsq, VW, SQ], f32, kind="ExternalOutput")

    with tile.TileContext(nc) as tc:
        if ablate:
            tc.race_detector_enabled = False
        pt_bufs = 1 + (stagger + 1) // 2
        with (
            tc.tile_pool(name="heads", bufs=hpc) as head_pool,
            tc.tile_pool(name="mask", bufs=nsq) as mask_pool,
            tc.tile_pool(name="pt", bufs=pt_bufs) as pt_pool,
            tc.tile_pool(name="tail", bufs=2) as tail_pool,
            tc.tile_pool(name="qk_ps", bufs=2, space="PSUM") as qk_pool,
            tc.tile_pool(name="o_ps", bufs=2, space="PSUM") as o_pool,
        ):
            # DMA order tracks first-use order in the compute stream (units
            # iterate sqb-major, h-minor): q/k of head h, then mask of sqb 0,
            # then V (first needed at the first PV, `stagger` halves in).
            qt_t, kt_t, vp_t = [], [], []
            for h in range(hpc):
                q_t = head_pool.tile([128, s], f16, name=f"qt_sb{h}", tag="qt")
                k_t = head_pool.tile([128, s], f16, name=f"kt_sb{h}", tag="kt")
                # Q^T/K^T live duplicated in both partition halves so the two
                # row-packed K=64 matmuls can run concurrently on the PE.
                nc.sync.dma_start(out=q_t[0:D, :], in_=qt_d[h, :, :])
                nc.sync.dma_start(out=q_t[D:128, :], in_=qt_d[h, :, :])
                nc.sync.dma_start(out=k_t[0:D, :], in_=kt_d[h, :, :])
                nc.sync.dma_start(out=k_t[D:128, :], in_=kt_d[h, :, :])
                qt_t.append(q_t)
                kt_t.append(k_t)

            # The whole 0/1 mask fits in SBUF — load it once, outside any
            # measurement loop (saves 8MB of DMA per pass).
            mk_t = {}     # sqb -> mask tile [128, nck*SQ] (chunk-major columns)
            for sqb in range(nsq):
                mk = mask_pool.tile([CK, nck * SQ], f16, name=f"mk_sb{sqb}",
                                    tag="mk")
                nc.sync.dma_start(out=mk[:, :], in_=mk_d[sqb, :, :])
                mk_t[sqb] = mk

            for h in range(hpc):
                v_t = head_pool.tile([CK, nck * VW], f16, name=f"vp_sb{h}", tag="vp")
                nc.sync.dma_start(out=v_t[:, :], in_=vp_d[h, :, :])
                vp_t.append(v_t)

            pt_t = {}     # (sqb, h) -> p^T tile [128, nck*SQ] fp16
            o_ps = {}     # (sqb, h) -> PSUM accumulator [VW, SQ]

            def emit_qk_group(sqb, h, c0, n, path):
                """QK matmuls + exp + keep-mask for chunks [c0, c0+n)."""
                qk = None
                if "qk" not in ablate:
                    qk = qk_pool.tile([128, n * SQ], f32,
                                      name=f"qk_{sqb}_{h}_{c0}", tag="qk",
                                      padded_shape=[128, 3 * SQ])
                for j in range(n):
                    if "qk" in ablate:
                        break
                    c = c0 + j
                    if qk128:
                        # Q^T/K^T live duplicated in both partition halves, so
                        # a single contract-128 matmul computes 2x the score
                        # in the same 512 cycles as a 64-contract one.  The
                        # doubling is folded into the exp scale.
                        nc.tensor.matmul(
                            qk[:, j * SQ:(j + 1) * SQ],
                            lhsT=kt_t[h][:, c * CK:(c + 1) * CK],
                            rhs=qt_t[h][:, sqb * SQ:(sqb + 1) * SQ],
                            start=True,
                            stop=True,
                        )
                    else:
                        # Row-group for PE packing: alternate on the global
                        # chunk index so consecutive matmuls never share
                        # row-groups.
                        bp = 64 * (c % 2)
                        nc.tensor.matmul(
                            qk[:, j * SQ:(j + 1) * SQ],
                            lhsT=kt_t[h][bp:bp + D, c * CK:(c + 1) * CK],
                            rhs=qt_t[h][bp:bp + D, sqb * SQ:(sqb + 1) * SQ],
                            start=True,
                            stop=True,
                            tile_position=(bp, 0),
                        )
                pt = pt_t[(sqb, h)]
                lo = c0 * SQ
                hi = (c0 + n) * SQ
                sc = 0.0625 if qk128 else 0.125
                if "act" not in ablate:
                    act_in = qk[:, :] if qk is not None else mk_t[sqb][:, lo:hi]
                    if path == "a":
                        nc.scalar.activation(pt[:, lo:hi], act_in, FT.Exp,
                                             scale=sc)
                    else:
                        nc.vector.tensor_scalar(
                            pt[:, lo:hi].bitcast(mybir.dt.int16),
                            act_in,
                            EXP_A * sc * 8,
                            EXP_B,
                            op0=mybir.AluOpType.mult,
                            op1=mybir.AluOpType.add,
                        )
                # Per-group mask keeps the exp->mask->PV chain fine-grained so
                # PV of a half never waits on a whole-half DVE pass.
                if "mask" not in ablate and pt is not None:
                    eng = nc.gpsimd if c0 >= 16 - n_gp else nc.vector
                    eng.tensor_tensor(
                        pt[:, lo:hi], pt[:, lo:hi], mk_t[sqb][:, lo:hi],
                        op=mybir.AluOpType.mult,
                    )

            def emit_pv(sqb, h, clo, chi):
                """PV matmuls for chunks [clo, chi), accumulating."""
                if "pv" in ablate:
                    return
                pt = pt_t[(sqb, h)]
                if "act" in ablate and "mask" in ablate:
                    pt = mk_t[sqb]  # stand-in written tile for PE-only ablations
                if clo == 0:
                    # Allocate at first use, not at emit_front: with deeper
                    # stagger an early allocation would hold PSUM banks for
                    # the whole front->back gap.
                    o_ps[(sqb, h)] = o_pool.tile(
                        [VW, SQ], f32, name=f"ops_{sqb}_{h}", tag="ops")
                ops = o_ps[(sqb, h)]
                for c in range(clo, chi):
                    nc.tensor.matmul(
                        ops[:, :],
                        lhsT=vp_t[h][:, c * VW:c * VW + VW],
                        rhs=pt[:, c * SQ:(c + 1) * SQ],
                        start=(c == 0),
                        stop=(c == nck - 1),
                    )

            def emit_tail(sqb, h):
                """Evacuate O^T' (unnormalized + Z row) and store."""
                if "tail" in ablate:
                    return
                ops = o_ps[(sqb, h)]
                ot = tail_pool.tile([VW, SQ], f32, name=f"ot_{sqb}_{h}", tag="ot")
                if tail_act:
                    nc.scalar.copy(ot[:, :], ops[:, :])
                else:
                    nc.vector.tensor_copy(ot[:, :], ops[:, :])
                nc.sync.dma_start(out=o_d[h, sqb, :, :], in_=ot[:, :])

            # Half-stage software pipeline over (sqb, h, half): half k's
            # QK/exp/mask stream overlaps the PV matmuls of half k-stagger,
            # so the in-order PE queue never stalls on the ACT/DVE work of
            # the half just emitted.
            ghalves = [(g0, g1) for _, g0, g1 in HALVES]
            if len(groups) != len(GROUPS):  # small-s debug builds: one half
                ghalves = [(0, len(groups))]

            def emit_front(sqb, h, hf):
                if hf == 0:
                    if not ({"act", "mask"} <= ablate):
                        pt_t[(sqb, h)] = pt_pool.tile(
                            [128, nck * SQ], f16, name=f"pt_{sqb}_{h}",
                            tag="pt")
                    else:
                        pt_t[(sqb, h)] = None
                g0, g1 = ghalves[hf]
                for c0, n, path in groups[g0:g1]:
                    emit_qk_group(sqb, h, c0, n, path)

            def emit_back(sqb, h, hf):
                g0, g1 = ghalves[hf]
                clo = groups[g0][0]
                chi = (groups[g1 - 1][0] + groups[g1 - 1][1])
                emit_pv(sqb, h, clo, chi)
                if hf == len(ghalves) - 1:
                    emit_tail(sqb, h)

            def emit_all():
                halves = [(sqb, h, hf)
                          for sqb in range(nsq) for h in range(hpc)
                          for hf in range(len(ghalves))]
                d = min(stagger, len(halves))
                for k, hv in enumerate(halves):
                    emit_front(*hv)
                    if k >= d:
                        emit_back(*halves[k - d])
                for hv in halves[-d:]:
                    emit_back(*hv)

            if loop_n is None:
                emit_all()
            else:
                hints = (mybir.EngineType.PE, mybir.EngineType.Activation,
                         mybir.EngineType.DVE)
                with tc.For_i(0, loop_n, 1, hint_engines=hints,
                              staggered_reset=bool(loop_stagger)):
                    emit_all()

    nc.finalize()
    return nc


def shard_inputs(K, Q, V, mask, hpc=HPC, s=S, n_cores=N_CORES):
    """Full inputs -> per-core in_maps with device-friendly host layouts."""
    nsq = s // SQ
    nck = s // CK
    n_units = n_cores * hpc
    Kf = np.asarray(K, np.float32).reshape(n_units, s, D)
    Qf = np.asarray(Q, np.float32).reshape(n_units, s, D)
    Vf = np.asarray(V, np.float32).reshape(n_units, s, D)
    keepT = (~np.asarray(mask).reshape(s, s)).T  # [k, q], True = attend
    mk_host = np.ascontiguousarray(
        keepT.astype(np.float16)
        .reshape(nck, CK, nsq, SQ)
        .transpose(2, 1, 0, 3)
        .reshape(nsq, CK, nck * SQ)
    )
    in_maps = []
    for c in range(n_cores):
        sl = slice(c * hpc, (c + 1) * hpc)
        qt = np.ascontiguousarray(Qf[sl].transpose(0, 2, 1)).astype(np.float16)
        kt = np.ascontiguousarray(Kf[sl].transpose(0, 2, 1)).astype(np.float16)
        vp = np.zeros((hpc, s, VW), np.float16)
        vp[:, :, :D] = Vf[sl]
        vp[:, :, D] = 1.0
        vp = np.ascontiguousarray(
            vp.reshape(hpc, nck, CK, VW).transpose(0, 2, 1, 3)
            .reshape(hpc, CK, nck * VW)
        )
        in_maps.append({"qt": qt, "kt": kt, "vp": vp, "mk": mk_host})
    return in_maps


_NC_CACHE = {}


def _get_nc():
    if "nc" not in _NC_CACHE:
        _NC_CACHE["nc"] = build_nc()
    return _NC_CACHE["nc"]


def run_sharded(in_maps, trace=False, **kwargs):
    return run_bass_kernel_spmd(
        _get_nc(), in_maps, core_ids=list(range(N_CORES)), trace=trace, **kwargs
    )


def unshard_output(per_core_raw, hpc=HPC, s=S):
    """[hpc, nsq, VW, SQ] raw blocks per core -> [n*hpc, s, D] normalized.

    Row D of each block is the softmax denominator Z; dividing and
    transposing here is O(S*D) host work (same order as unsharding).
    """
    n = len(per_core_raw)
    out = np.empty((n * hpc, s, D), np.float32)
    for c, o in enumerate(per_core_raw):
        ot = o[:, :, :D, :] / o[:, :, D:D + 1, :]   # [hpc, nsq, D, SQ]
        out[c * hpc:(c + 1) * hpc] = (
            ot.transpose(0, 1, 3, 2).reshape(hpc, s, D))
    return out


def assemble_output(results):
    out = unshard_output([results[c]["o"] for c in range(N_CORES)])
    return out.reshape(B, H, S, D)


def kernel(K, Q, V, mask):
    in_maps = shard_inputs(K, Q, V, mask)
    res = run_sharded(in_maps)
    return assemble_output(res.results)

